# revision 1
# baseline (speedup 1.0000x reference)
"""Trainium2 Bass kernel for per-sample masked conv2d (dynamic weight attention conv).

out[b] = conv2d(x[b], weight * m[b], stride=1, pad=1) + bias

Strategy: pure data parallel over batch (32 samples -> 8 cores x 4 samples).
Per sample, the conv is computed as 9 shifted matmuls accumulated in PSUM:
  out[o, h, w] = sum_{kh,kw,i} mw[o,i,kh,kw] * xpad[i, h+kh, w+kw]
with mw = weight * m[b].  The masked weights are produced in natural [o, (i kh kw)]
layout by a DVE elementwise multiply, then transposed 128x128-tile-wise on the
TensorEngine into the [i, o] layout the matmul's stationary operand needs.
Matmuls run as float32r (full-rate fp32 path, N=448 >= 256).
"""

import sys
from contextlib import ExitStack

for _p in ("/opt/trn_rl_repo",):
    if _p not in sys.path:
        sys.path.append(_p)

import numpy as np

import concourse.bass as bass
import concourse.mybir as mybir
import concourse.tile as tile
from concourse import bacc, bass_utils
from concourse.masks import make_identity

# Enable walrus LDWEIGHTS dedup: consecutive matmuls sharing the same
# stationary weights then skip the redundant fp32 weight reload, which is
# what paces the PE otherwise.  Patch the flag at the run_command boundary.
if not getattr(bass_utils, "_ldw_opt_patched", False):
    _orig_run_command = bass_utils.run_command

    def _run_command_ldw(argv, **kwargs):
        argv = ["--enable-ldw-opt=true" if a == "--enable-ldw-opt=false" else a
                for a in argv]
        return _orig_run_command(argv, **kwargs)

    bass_utils.run_command = _run_command_ldw
    bass_utils._ldw_opt_patched = True

# Problem constants (hardcoded per contract)
B, FIN, FOUT, KK, H, W = 32, 256, 256, 3, 56, 56
N_CORES = 8
BPC = B // N_CORES          # samples per core = 4
P = 128                     # partition width
NI = FIN // P               # input-channel chunks = 2
NO = FOUT // P              # output-channel chunks = 2
HP, WP = H + 2, W + 2       # padded spatial = 58x58
RG_ROWS = 8                 # output rows per matmul group
NRG = H // RG_ROWS          # row groups = 7
NTILE = RG_ROWS * W         # moving free size = 448 (<=512 fp32, >=256 for f32r)
F32 = mybir.dt.float32
F32R = mybir.dt.float32r


def build_program():
    """Build the single-core Bass program (same program on all 8 cores)."""
    nc = bacc.Bacc("TRN2", target_bir_lowering=False, debug=False,
                   num_devices=N_CORES)

    x_d = nc.dram_tensor("x", [BPC, FIN, H, W], F32, kind="ExternalInput").ap()
    m_d = nc.dram_tensor("m", [BPC, FOUT, FIN, KK, KK], F32,
                         kind="ExternalInput").ap()
    w_d = nc.dram_tensor("weight", [FOUT, FIN, KK, KK], F32,
                         kind="ExternalInput").ap()
    b_d = nc.dram_tensor("bias", [FOUT], F32, kind="ExternalInput").ap()
    o_d = nc.dram_tensor("out", [BPC, FOUT, H, W], F32,
                         kind="ExternalOutput").ap()

    KSQ = KK * KK                      # 9
    CFREE = FIN * KSQ                  # 2304: (i kh kw) flattened

    with tile.TileContext(nc) as tc, ExitStack() as ctx:
        consts = ctx.enter_context(tc.tile_pool(name="consts", bufs=1))
        m_pool = ctx.enter_context(tc.tile_pool(name="m_pool", bufs=NO))
        mw_pool = ctx.enter_context(tc.tile_pool(name="mw_pool", bufs=NO))
        xs_pool = ctx.enter_context(tc.tile_pool(name="xs_pool", bufs=2))
        xp_pool = ctx.enter_context(tc.tile_pool(name="xp_pool", bufs=2 * NI))
        wt_pool = ctx.enter_context(tc.tile_pool(name="wt_pool",
                                                 bufs=NO * NI * KSQ))
        out_pool = ctx.enter_context(tc.tile_pool(name="out_pool", bufs=2))
        acc_psum = ctx.enter_context(tc.tile_pool(name="acc_psum", bufs=5,
                                                  space="PSUM"))
        tp_psum = ctx.enter_context(tc.tile_pool(name="tp_psum", bufs=3,
                                                 space="PSUM"))

        # --- per-core constants (loaded on the gpsimd/SWDGE ring so they
        # don't serialize with the per-sample m/out loads on the sync ring) ---
        ident = consts.tile([P, P], F32, name="ident")
        make_identity(nc, ident)
        ident_r = consts.tile([P, P], F32R, name="ident_r")
        nc.vector.tensor_copy(ident_r, ident)

        # weight in natural layout: [o_chunk][128, (i kh kw)]
        # (w1/bias loads are emitted after sample 0's x loads so the gpsimd
        # DMA ring serves the critical path first)
        w_nat = w_d.rearrange("(c p) i kh kw -> c p (i kh kw)", p=P)
        w_tiles = []
        for oc in range(NO):
            wt = consts.tile([P, CFREE], F32, name=f"w_nat_{oc}", tag=f"w{oc}")
            w_tiles.append(wt)
        WH = CFREE // NI
        nc.gpsimd.dma_start(out=w_tiles[0][:, :WH], in_=w_nat[0][:, :WH])
        nc.gpsimd.dma_start(out=w_tiles[0][:, WH:], in_=w_nat[0][:, WH:])

        # bias: [128, NO] with bias_t[p, oc] = bias[oc*128 + p]
        bias_t = consts.tile([P, NO], F32, name="bias_t")

        x_nat = x_d.rearrange("s (c p) h w -> s c p h w", p=P)
        m_nat = m_d.rearrange("s (c p) i kh kw -> s c p (i kh kw)", p=P)
        o_nat = o_d.rearrange("s (c p) h w -> s c p (h w)", p=P)

        for s in range(BPC):
            # --- masked weights in natural layout, rounded to f32r so the
            # PE transposes can run at the faster f32r rate; m is loaded in
            # per-ic halves so the first transposes can start early ---
            mw_tiles = []
            xp_tiles = []
            HALF = CFREE // NI

            def load_m(oc):
                mt = m_pool.tile([P, CFREE], F32, name=f"m_{s}_{oc}", tag="m")
                for h in range(NI):
                    nc.sync.dma_start(out=mt[:, h * HALF:(h + 1) * HALF],
                                      in_=m_nat[s, oc][:, h * HALF:(h + 1) * HALF])
                mw = mw_pool.tile([P, CFREE], F32R, name=f"mw_{s}_{oc}",
                                  tag="mw")
                for h in range(NI):
                    sl = slice(h * HALF, (h + 1) * HALF)
                    nc.vector.tensor_mul(mw[:, sl], mt[:, sl],
                                         w_tiles[oc][:, sl])
                mw_tiles.append(mw)

            def load_x(ic):
                # staging tile carries a 64-elem zero scratch at the end; all
                # xp writes are DVE copies (memset can't emit f32r); DMA is
                # contiguous for efficient descriptors, repack+round on DVE.
                xs = xs_pool.tile([P, H * W + 64], F32, name=f"xs_{s}_{ic}",
                                  tag="xs")
                nc.vector.memset(xs[:, H * W:], 0.0)
                RH = H // 2
                nc.gpsimd.dma_start(out=xs[:, :RH * W],
                                    in_=x_nat[s, ic][:, :RH, :])
                nc.gpsimd.dma_start(out=xs[:, RH * W:H * W],
                                    in_=x_nat[s, ic][:, RH:, :])
                xp = xp_pool.tile([P, HP, WP], F32R, name=f"xp_{s}_{ic}",
                                  tag="xp")
                z = xs[:, H * W:H * W + WP]
                nc.vector.tensor_copy(xp[:, 0, :], z)
                nc.vector.tensor_copy(xp[:, HP - 1, :], z)
                zc = xs[:, H * W:H * W + H].rearrange("p (h o) -> p h o", o=1)
                nc.vector.tensor_copy(xp[:, 1:HP - 1, 0:1], zc)
                nc.vector.tensor_copy(xp[:, 1:HP - 1, WP - 1:WP], zc)
                nc.vector.tensor_copy(
                    xp[:, 1:RH + 1, 1:WP - 1],
                    xs[:, :RH * W].rearrange("p (h w) -> p h w", w=W))
                nc.vector.tensor_copy(
                    xp[:, RH + 1:HP - 1, 1:WP - 1],
                    xs[:, RH * W:H * W].rearrange("p (h w) -> p h w", w=W))
                xp_tiles.append(xp)

            load_m(0)
            load_x(0)
            if s == 0:
                # stream the remaining constants behind sample 0's first loads
                # (must precede load_m(1), whose multiply reads w_tiles[1])
                nc.gpsimd.dma_start(out=w_tiles[1], in_=w_nat[1])
                nc.gpsimd.dma_start(out=bias_t,
                                    in_=b_d.rearrange("(c p) -> p c", p=P))
            load_m(1)
            load_x(1)

            # --- transpose masked weights into [i, o] stationary tiles ---
            # mwT[oc][ic][k][i_part, o_free] = mw[o, i, kh, kw]
            mwT = [[[None] * KSQ for _ in range(NI)] for _ in range(NO)]
            for oc in range(NO):
                mw3 = mw_tiles[oc].rearrange("p (i k) -> p i k", k=KSQ)
                for ic in range(NI):
                    for k in range(KSQ):
                        tp = tp_psum.tile([P, P], F32R,
                                          name=f"tp_{s}_{oc}_{ic}_{k}", tag="tp")
                        nc.tensor.transpose(tp, mw3[:, ic * P:(ic + 1) * P, k],
                                            ident_r)
                        wt = wt_pool.tile([P, P], F32R,
                                          name=f"mwT_{s}_{oc}_{ic}_{k}", tag="mwT")
                        nc.vector.tensor_copy(wt, tp)
                        mwT[oc][ic][k] = wt

            # --- conv matmuls ---
            # rowgroups are processed in blocks of up to 3 sharing the same
            # stationary weights on consecutive matmuls, so walrus's ldw-opt
            # can skip redundant fp32 LDWEIGHTS (the PE pitch limiter).
            n_mm = KSQ * NI
            for oc in range(NO):
                osb = out_pool.tile([P, H * W], F32, name=f"osb_{s}_{oc}",
                                    tag="osb")
                for block in ((0, 1, 2), (3, 4, 5), (6,)):
                    accs = {rg: acc_psum.tile([P, NTILE], F32,
                                              name=f"acc_{s}_{oc}_{rg}",
                                              tag="acc")
                            for rg in block}
                    for idx in range(n_mm):
                        ic, k = divmod(idx, KSQ)
                        kh, kw = divmod(k, KK)
                        for rg in block:
                            r0 = rg * RG_ROWS + kh
                            rhs = xp_tiles[ic][:, r0:r0 + RG_ROWS, kw:kw + W]
                            nc.tensor.matmul(
                                accs[rg],
                                mwT[oc][ic][k],
                                rhs,
                                start=(idx == 0),
                                stop=(idx == n_mm - 1),
                            )
                    for rg in block:
                        # drain PSUM -> SBUF with bias add (Identity act)
                        nc.scalar.add(osb[:, rg * NTILE:(rg + 1) * NTILE],
                                      accs[rg], bias_t[:, oc:oc + 1])
                    # stream the output per block so the final DMA mostly
                    # hides under remaining matmuls
                    lo, hi = block[0] * NTILE, (block[-1] + 1) * NTILE
                    hi = min(hi, H * W)
                    nc.sync.dma_start(out=o_nat[s, oc][:, lo:hi],
                                      in_=osb[:, lo:hi])

    nc.compile()
    return nc


def shard_inputs(x, m, weight, bias):
    """Split batch across cores; replicate weight/bias."""
    x = np.ascontiguousarray(np.asarray(x, dtype=np.float32))
    m = np.ascontiguousarray(np.asarray(m, dtype=np.float32))
    weight = np.ascontiguousarray(np.asarray(weight, dtype=np.float32))
    bias = np.ascontiguousarray(np.asarray(bias, dtype=np.float32))
    in_maps = []
    for c in range(N_CORES):
        sl = slice(c * BPC, (c + 1) * BPC)
        in_maps.append({"x": x[sl], "m": m[sl], "weight": weight, "bias": bias})
    return in_maps


def kernel(x, m, weight, bias, _trace=False):
    nc = build_program()
    in_maps = shard_inputs(x, m, weight, bias)
    res = bass_utils.run_bass_kernel_spmd(
        nc, in_maps, core_ids=list(range(N_CORES)), trace=_trace
    )
    out = np.concatenate([res.results[c]["out"] for c in range(N_CORES)], axis=0)
    if _trace:
        kernel.last_results = res
    return out



# revision 4
# speedup vs baseline: 1.0765x; 1.0765x over previous
"""Trainium2 Bass kernel for per-sample masked conv2d (dynamic weight attention conv).

out[b] = conv2d(x[b], weight * m[b], stride=1, pad=1) + bias

Strategy: pure data parallel over batch (32 samples -> 8 cores x 4 samples).
Per sample the conv runs as 18 accumulation stages (2 input-channel chunks x 9
taps) of matmuls over 7 row-group PSUM accumulators, so each stationary weight
load serves 7 consecutive matmuls.  The datapath is bf16: masked weights are
built by a DVE multiply (f32 m x f32 w -> bf16), transposed 128x128-tile-wise
on the TensorEngine into [i, o] stationary layout, packed 8-per-PSUM-bank and
drained by the Activation engine.  Transposes for sample s+1 are interleaved
between sample s's matmul stages so the PE never idles.
"""

import sys
from contextlib import ExitStack

for _p in ("/opt/trn_rl_repo",):
    if _p not in sys.path:
        sys.path.append(_p)

import numpy as np

import concourse.bass as bass
import concourse.mybir as mybir
import concourse.tile as tile
from concourse import bacc, bass_utils
from concourse.masks import make_identity

# NOTE: walrus --enable-ldw-opt rejects the standalone InstLdweights that
# bass emits for non-f32 stationary dtypes, so it stays at its default
# (false) for this bf16 kernel.

# Problem constants (hardcoded per contract)
B, FIN, FOUT, KK, H, W = 32, 256, 256, 3, 56, 56
N_CORES = 8
BPC = B // N_CORES          # samples per core = 4
P = 128                     # partition width
NI = FIN // P               # input-channel chunks = 2
NO = FOUT // P              # output-channel chunks = 2
HP, WP = H + 2, W + 2       # padded spatial = 58x58
RG = 8                      # output rows per row-group
NRG = H // RG               # row groups = 7
NT = RG * W                 # matmul moving free size = 448
KSQ = KK * KK               # 9
CFREE = FIN * KSQ           # 2304
HALF = CFREE // NI          # 1152
NST = NO * NI * KSQ         # 36 weight stages per sample
RH = H // 2                 # 28
F32 = mybir.dt.float32
BF16 = mybir.dt.bfloat16

# transpose groups: stages [t0, t1) packed into one PSUM bank per group
TP_GROUPS = [(0, 8), (8, 16), (16, 24), (24, 32), (32, 36)]


def build_program():
    nc = bacc.Bacc("TRN2", target_bir_lowering=False, debug=False,
                   num_devices=N_CORES)

    x_d = nc.dram_tensor("x", [BPC, FIN, H, W], F32, kind="ExternalInput").ap()
    m_d = nc.dram_tensor("m", [BPC, FOUT, FIN, KK, KK], F32,
                         kind="ExternalInput").ap()
    w_d = nc.dram_tensor("weight", [FOUT, FIN, KK, KK], F32,
                         kind="ExternalInput").ap()
    b_d = nc.dram_tensor("bias", [FOUT], F32, kind="ExternalInput").ap()
    o_d = nc.dram_tensor("out", [BPC, FOUT, H, W], F32,
                         kind="ExternalOutput").ap()

    x_nat = x_d.rearrange("s (c p) h w -> s c p h w", p=P)
    m_nat = m_d.rearrange("s (c p) i kh kw -> s c p (i kh kw)", p=P)
    w_nat = w_d.rearrange("(c p) i kh kw -> c p (i kh kw)", p=P)
    o_nat = o_d.rearrange("s (c p) h w -> s c p (h w)", p=P)

    with tile.TileContext(nc) as tc, ExitStack() as ctx:
        consts = ctx.enter_context(tc.tile_pool(name="consts", bufs=1))
        m_pool = ctx.enter_context(tc.tile_pool(name="m_pool", bufs=2))
        mw_pool = ctx.enter_context(tc.tile_pool(name="mw_pool", bufs=2))
        xs_pool = ctx.enter_context(tc.tile_pool(name="xs_pool", bufs=2))
        xp_pool = ctx.enter_context(tc.tile_pool(name="xp_pool", bufs=2 * NI))
        wt_pool = ctx.enter_context(tc.tile_pool(name="wt_pool", bufs=2))
        osb_pool = ctx.enter_context(tc.tile_pool(name="osb_pool", bufs=2))
        acc_psum = ctx.enter_context(tc.tile_pool(name="acc_psum", bufs=NRG,
                                                  space="PSUM"))
        tp_psum = ctx.enter_context(tc.tile_pool(name="tp_psum", bufs=1,
                                                 space="PSUM"))

        ident = consts.tile([P, P], F32, name="ident")
        make_identity(nc, ident)
        ident_b = consts.tile([P, P], BF16, name="ident_b")
        nc.vector.tensor_copy(ident_b, ident)

        # weight, natural layout f32 [oc][128, (i kh kw)]; on the gpsimd ring
        w_tiles = [consts.tile([P, CFREE], F32, name=f"w_{oc}")
                   for oc in range(NO)]
        bias_t = consts.tile([P, NO], F32, name="bias_t")
        for oc in range(NO):
            for h in range(NI):
                sl = slice(h * HALF, (h + 1) * HALF)
                nc.gpsimd.dma_start(out=w_tiles[oc][:, sl],
                                    in_=w_nat[oc][:, sl])
        nc.gpsimd.dma_start(out=bias_t,
                            in_=b_d.rearrange("(c p) -> p c", p=P))

        # per-sample state
        mw_tiles = {}   # s -> [oc] bf16 [P, CFREE]
        mwT = {}        # s -> bf16 [P, NST*P] stationary store
        xp_tiles = {}   # s -> [ic] bf16 [P, HP, WP]

        def stage_params(t):
            oc, r = divmod(t, NI * KSQ)
            ic, k = divmod(r, KSQ)
            kh, kw = divmod(k, KK)
            return oc, ic, kh, kw

        def emit_m_loads(s):
            mts = []
            for oc in range(NO):
                mt = m_pool.tile([P, CFREE], F32, name=f"m_{s}_{oc}", tag="m")
                for h in range(NI):
                    sl = slice(h * HALF, (h + 1) * HALF)
                    nc.sync.dma_start(out=mt[:, sl], in_=m_nat[s, oc][:, sl])
                mts.append(mt)
            return mts

        def emit_x_loads(s):
            xss = []
            for ic in range(NI):
                xs = xs_pool.tile([P, H * W], F32, name=f"xs_{s}_{ic}",
                                  tag="xs")
                nc.scalar.dma_start(out=xs[:, :RH * W],
                                    in_=x_nat[s, ic][:, :RH, :])
                nc.scalar.dma_start(out=xs[:, RH * W:],
                                    in_=x_nat[s, ic][:, RH:, :])
                xss.append(xs)
            return xss

        def emit_mul(s, mts, oc):
            mw = mw_pool.tile([P, CFREE], BF16, name=f"mw_{s}_{oc}", tag="mw")
            for h in range(NI):
                sl = slice(h * HALF, (h + 1) * HALF)
                nc.vector.tensor_mul(mw[:, sl], mts[oc][:, sl],
                                     w_tiles[oc][:, sl])
            mw_tiles.setdefault(s, []).append(mw)

        def emit_xp_borders(s, ic, xp):
            # zero halo via gpsimd (idle engine); interior overwritten later
            nc.gpsimd.memset(xp[:, 0, :], 0.0)
            nc.gpsimd.memset(xp[:, HP - 1, :], 0.0)
            nc.gpsimd.memset(xp[:, 1:HP - 1, 0:1], 0.0)
            nc.gpsimd.memset(xp[:, 1:HP - 1, WP - 1:WP], 0.0)

        def emit_xp_alloc(s, ic):
            xp = xp_pool.tile([P, HP, WP], BF16, name=f"xp_{s}_{ic}", tag="xp")
            xp_tiles.setdefault(s, {})[ic] = xp
            emit_xp_borders(s, ic, xp)
            return xp

        def emit_xp_interior(s, ic, xss, half, eng):
            xp = xp_tiles[s][ic]
            copy = eng.copy if eng is nc.scalar else eng.tensor_copy
            if half == 0:
                copy(xp[:, 1:RH + 1, 1:WP - 1],
                     xss[ic][:, :RH * W].rearrange("p (h w) -> p h w", w=W))
            else:
                copy(xp[:, RH + 1:HP - 1, 1:WP - 1],
                     xss[ic][:, RH * W:].rearrange("p (h w) -> p h w", w=W))

        def emit_wt_alloc(s):
            mwT[s] = wt_pool.tile([P, NST * P], BF16, name=f"mwT_{s}",
                                  tag="mwT")

        def emit_tp_group(s, gi):
            t0, t1 = TP_GROUPS[gi]
            n = t1 - t0
            tp = tp_psum.tile([P, 8 * P], BF16, name=f"tp_{s}_{gi}", tag="tp")
            for j, t in enumerate(range(t0, t1)):
                oc, ic, kh, kw = stage_params(t)
                k = kh * KK + kw
                mw3 = mw_tiles[s][oc].rearrange("p (i k) -> p i k", k=KSQ)
                nc.tensor.transpose(tp[:, j * P:(j + 1) * P],
                                    mw3[:, ic * P:(ic + 1) * P, k], ident_b)
            # drain the packed bank to the stationary store on Act
            nc.scalar.copy(mwT[s][:, t0 * P:t1 * P], tp[:, :n * P])

        def emit_sample_compute(s, interleave):
            """36 weight stages; 7 matmuls each into per-rowgroup PSUM accs.

            interleave: {stage_idx: [callable]} emitted after that stage.
            """
            accs = None
            for t in range(NST):
                oc, ic, kh, kw = stage_params(t)
                local = t % (NI * KSQ)
                if local == 0:
                    accs = [acc_psum.tile([P, NT], F32,
                                          name=f"acc_{s}_{oc}_{rg}", tag="acc")
                            for rg in range(NRG)]
                for rg in range(NRG):
                    r0 = rg * RG + kh
                    rhs = xp_tiles[s][ic][:, r0:r0 + RG, kw:kw + W]
                    nc.tensor.matmul(accs[rg], mwT[s][:, t * P:(t + 1) * P],
                                     rhs, start=(local == 0),
                                     stop=(local == NI * KSQ - 1))
                if local == NI * KSQ - 1:
                    osb = osb_pool.tile([P, H * W], F32, name=f"osb_{s}_{oc}",
                                        tag="osb")
                    for rg in range(NRG):
                        sl = slice(rg * NT, (rg + 1) * NT)
                        nc.scalar.add(osb[:, sl], accs[rg],
                                      bias_t[:, oc:oc + 1])
                        nc.sync.dma_start(out=o_nat[s, oc][:, sl],
                                          in_=osb[:, sl])
                for fn in interleave.get(t, []):
                    fn()

        # ---------------- prologue: sample 0 ----------------
        mts0 = emit_m_loads(0)
        xss0 = emit_x_loads(0)
        emit_mul(0, mts0, 0)                      # DVE: mw[0][oc0]
        emit_wt_alloc(0)
        emit_xp_alloc(0, 0)
        emit_xp_alloc(0, 1)
        emit_tp_group(0, 0)                       # PE tps 0-7, drain on Act
        emit_tp_group(0, 1)                       # PE tps 8-15
        emit_xp_interior(0, 0, xss0, 0, nc.vector)
        emit_xp_interior(0, 0, xss0, 1, nc.vector)
        emit_mul(0, mts0, 1)                      # DVE: mw[0][oc1]
        emit_xp_interior(0, 1, xss0, 0, nc.scalar)
        emit_xp_interior(0, 1, xss0, 1, nc.scalar)

        pending = {}   # interleave map for the current sample's compute

        def add_il(t, fn):
            pending.setdefault(t, []).append(fn)

        # remaining transpose groups of sample 0 interleave into early stages
        add_il(1, lambda: emit_tp_group(0, 2))
        add_il(4, lambda: emit_tp_group(0, 3))
        add_il(7, lambda: emit_tp_group(0, 4))

        for s in range(BPC):
            nxt = s + 1
            if nxt < BPC:
                # next-sample loads + weight production, emitted at s's top
                mts = emit_m_loads(nxt)
                xss = emit_x_loads(nxt)
                emit_mul(nxt, mts, 0)
                emit_mul(nxt, mts, 1)
                emit_wt_alloc(nxt)
                emit_xp_alloc(nxt, 0)
                emit_xp_alloc(nxt, 1)
                emit_xp_interior(nxt, 0, xss, 0, nc.vector)
                emit_xp_interior(nxt, 0, xss, 1, nc.vector)
                # transposes of s+1 interleave into s's oc1 stages
                add_il(19, lambda s_=nxt: emit_tp_group(s_, 0))
                add_il(20, lambda s_=nxt, x_=xss: emit_xp_interior(
                    s_, 1, x_, 0, nc.scalar))
                add_il(22, lambda s_=nxt: emit_tp_group(s_, 1))
                add_il(23, lambda s_=nxt, x_=xss: emit_xp_interior(
                    s_, 1, x_, 1, nc.scalar))
                add_il(25, lambda s_=nxt: emit_tp_group(s_, 2))
                add_il(28, lambda s_=nxt: emit_tp_group(s_, 3))
                add_il(31, lambda s_=nxt: emit_tp_group(s_, 4))
            emit_sample_compute(s, pending)
            pending = {}

    nc.compile()
    return nc


def shard_inputs(x, m, weight, bias):
    """Split batch across cores; replicate weight/bias."""
    x = np.ascontiguousarray(np.asarray(x, dtype=np.float32))
    m = np.ascontiguousarray(np.asarray(m, dtype=np.float32))
    weight = np.ascontiguousarray(np.asarray(weight, dtype=np.float32))
    bias = np.ascontiguousarray(np.asarray(bias, dtype=np.float32))
    in_maps = []
    for c in range(N_CORES):
        sl = slice(c * BPC, (c + 1) * BPC)
        in_maps.append({"x": x[sl], "m": m[sl], "weight": weight, "bias": bias})
    return in_maps


def kernel(x, m, weight, bias, _trace=False):
    nc = build_program()
    in_maps = shard_inputs(x, m, weight, bias)
    res = bass_utils.run_bass_kernel_spmd(
        nc, in_maps, core_ids=list(range(N_CORES)), trace=_trace
    )
    out = np.concatenate([res.results[c]["out"] for c in range(N_CORES)], axis=0)
    if _trace:
        kernel.last_results = res
    return out
